# revision 17
# baseline (speedup 1.0000x reference)
"""Trainium2 kernel for 4096x4096 single-channel 7x7 valid cross-correlation + bias.

Strategy (v7): 64x32 PE tiling, kernel-column pairs in K, balanced DMA pipeline
------------------------------------------------------------------------------
HW model (probe-measured): tiled matmul throughput is bound by the serialized
LDWEIGHTS stream (~cols/1.2GHz + overhead per matmul, 1:1 LDW:MM, no elision);
each dma_start costs ~0.55us+transfer of issuing-engine time; DMA access
patterns support one partition-level dim only (free dims may nest).

- Strips of 32 input rows -> 26 output rows; K=64 packs TWO kernel columns
  per matmul: partitions [0:32) of each half hold strip rows, [32:64) the
  same rows shifted one column (one flat on-chip shift copy per quadrant).
  A zeroed pad column at 0 keeps all matmul reads on defined data.
  lhsT [64, 26] = stacked Toeplitz bands [T_{2js-1}; T_{2js}] (T_{-1}=0);
  4 matmuls per strip; 8 concurrent 64x32 PE tiles; 640 matmuls total with
  26-column weight loads (~2.2us per 64-matmul round).
- Round = 16 strips: quadrant r2 holds strips s = 16R+8r2+k in free slots k;
  strip (r2, g, c) -> PE tile (64r2, 32c), PSUM bank 2r2+g partitions 32c.
- Engine budget per round: sync = 2 input loads; scalar = shift q0->q1 +
  2 drains; gpsimd = shift q2->q3 + stores; vector = 2 drains.  Outputs
  pair-packed: two rounds share one [128, 8KB] staging tile, 4 stores/pair.
  3-round input prefetch keeps the PE fed.

Sharding: output columns across 8 cores (512 each + 6-col halo host-side).
"""

import os

import numpy as np
import ml_dtypes

import concourse.bass as bass
import concourse.bacc as bacc_mod
import concourse.mybir as mybir
import concourse.tile as tile
from concourse.bass_utils import run_bass_kernel_spmd

H = 4096          # input rows
W = 4096          # input cols
KH = 7            # kernel rows
KW = 7            # kernel cols
OH = H - KH + 1   # 4090 output rows
OW = W - KW + 1   # 4090 output cols
NCORES = 8
CW = 512          # output cols per core
SW = CW + KW - 1  # 518 input cols per shard

TS = 32           # input rows per strip
SOUT = TS - KH + 1  # 26 output rows per strip
NJS = 4           # matmuls per strip (kernel-column pairs)
NROUNDS = 10      # 16 strips per round
NSTRIPS = 16 * NROUNDS          # 160 strip slots (158 real)
FREEW = 4160                    # pad col + 8 strip slots + tail pad,
                                # 64-byte-aligned partition lines
PAD_ROWS = SOUT * (NSTRIPS - 1) + TS  # 4166

_BF16 = ml_dtypes.bfloat16


def _build_program(bias_val: float) -> bass.Bass:
    nc = bacc_mod.Bacc("TRN2", target_bir_lowering=False)

    x_d = nc.dram_tensor("xs", [NROUNDS, 2, TS, FREEW], mybir.dt.bfloat16,
                         kind="ExternalInput")
    w_d = nc.dram_tensor("tmat", [128, NJS * SOUT], mybir.dt.bfloat16,
                         kind="ExternalInput")
    # y[P, a, q, rr, b, w]: strip 16*(2P+rr)+4b+a, output row q, col w
    y_d = nc.dram_tensor("y", [NROUNDS // 2, 4, SOUT, 2, 4, CW],
                         mybir.dt.bfloat16, kind="ExternalOutput")

    with tile.TileContext(nc) as tc:
        with (
            tc.tile_pool(name="const", bufs=1) as constp,
            tc.tile_pool(name="xg", bufs=NROUNDS) as xgp,
            tc.tile_pool(name="yg", bufs=2) as ygp,
            tc.tile_pool(name="ps", bufs=8, space="PSUM") as psp,
        ):
            w_sb = constp.tile([128, NJS * SOUT], mybir.dt.bfloat16)
            nc.sync.dma_start(w_sb[:, :], w_d[:, :])

            xg_tiles = {}

            def load_round(R):
                xg = xgp.tile([128, FREEW], mybir.dt.bfloat16,
                              name="xg", tag="xg")
                xg_tiles[R] = xg
                nc.sync.dma_start(xg[0:32, :], x_d[R, 0])
                nc.sync.dma_start(xg[64:96, :], x_d[R, 1])
                # upper 32 of each quadrant = lower shifted 4 columns
                # (8-byte aligned source keeps the DMA at full rate)
                nc.scalar.dma_start(xg[32:64, 0:FREEW - 4], xg[0:32, 4:FREEW])
                nc.gpsimd.dma_start(xg[96:128, 0:FREEW - 4], xg[64:96, 4:FREEW])

            load_round(0)

            yt_pair = None
            for R in range(NROUNDS):
                # emit next round's loads first so each engine's FIFO
                # interleaves (load R+1 | mm R | drain R | store R)
                if R + 1 < NROUNDS:
                    load_round(R + 1)
                xg = xg_tiles.pop(R)
                # bank b = 2*r2 + g holds strips (R, r2, g, c=0..3) at
                # partition slice 32c; strip index s = 16R + 4b + c.
                ps_tiles = [psp.tile([128, CW], mybir.dt.float32,
                                     name="ps", tag="ps")
                            for _ in range(4)]

                for js in range(NJS):
                    for g in range(2):
                        for r2 in range(2):
                            b = 2 * r2 + g
                            for c in range(4):
                                k = 4 * g + c
                                off = 1 + k * SW + js
                                nc.tensor.matmul(
                                    ps_tiles[b][32 * c:32 * c + SOUT, :],
                                    w_sb[64 * r2:64 * r2 + 64,
                                         SOUT * js:SOUT * js + SOUT],
                                    xg[64 * r2:64 * r2 + 64, off:off + CW],
                                    start=(js == 0),
                                    stop=(js == NJS - 1),
                                    tile_position=(64 * r2, 32 * c),
                                )

                rr = R % 2
                if rr == 0:
                    yt_pair = ygp.tile([128, 8 * CW], mybir.dt.bfloat16,
                                       name="yg", tag="yg")
                for b in range(4):
                    dst = yt_pair[:, (4 * rr + b) * CW:(4 * rr + b + 1) * CW]
                    src = ps_tiles[b][:, :]
                    if b % 2 == 0:
                        nc.scalar.activation(
                            dst, src, mybir.ActivationFunctionType.Copy,
                            bias=float(bias_val),
                        )
                    else:
                        nc.vector.tensor_scalar_add(dst, src, float(bias_val))

                if rr == 1:
                    for a in range(4):
                        nc.gpsimd.dma_start(
                            y_d[R // 2, a],
                            yt_pair[32 * a:32 * a + SOUT, :],
                        )

    nc.compile()
    nc.finalize()
    return nc


def _toeplitz(weight: np.ndarray) -> np.ndarray:
    """[128, 4*26] bf16: block js holds stacked bands [T_js; T_{js+4}]
    (T_7 = 0), T_j[u, m] = W[u-m, j]; replicated for quadrant r2=1."""
    t = np.zeros((64, NJS * SOUT), np.float32)
    for js in range(NJS):
        for half in range(2):
            j = js + 4 * half
            if j >= KW:
                continue
            for i in range(KH):
                mm = np.arange(0, SOUT)
                t[32 * half + mm + i, js * SOUT + mm] = weight[i, j]
    return np.tile(t, (2, 1)).astype(_BF16)


def _pack_shard(x_bf: np.ndarray, c0: int) -> np.ndarray:
    """[10, 2, 32, FREEW] bf16: quadrant r2 of round R holds strips
    s = 16R + 8*r2 + k in free slot k (cols 1+k*SW), pad col 0 zero."""
    valid = min(SW, W - c0)
    xs = np.zeros((PAD_ROWS, SW), _BF16)
    xs[:H, :valid] = x_bf[:, c0:c0 + valid]
    R = np.arange(NROUNDS)
    out = np.zeros((NROUNDS, 2, TS, FREEW), _BF16)
    for r2 in range(2):
        for k in range(8):
            s = 16 * R + 8 * r2 + k
            rows = SOUT * s[:, None] + np.arange(TS)[None, :]
            out[:, r2, :, 1 + k * SW:1 + (k + 1) * SW] = xs[rows]
    return out


def _unpack_out(y_packed: np.ndarray) -> np.ndarray:
    """[5, 4, 26, 2, 4, 512] bf16 -> [4090, 512] f32 (strip 16R+4b+a)."""
    y = y_packed.reshape(NROUNDS // 2, 4, SOUT, 2, 4, CW)
    y = y.transpose(0, 3, 4, 1, 2, 5)   # [P, rr, b, a, q, w]
    return y.reshape(NSTRIPS * SOUT, CW)[:OH].astype(np.float32)


def kernel(x: np.ndarray, weight: np.ndarray, bias: np.ndarray) -> np.ndarray:
    x = np.asarray(x, dtype=np.float32)
    weight = np.asarray(weight, dtype=np.float32)
    bias = np.asarray(bias, dtype=np.float32)

    tmat = _toeplitz(weight)
    x_bf = x.astype(_BF16)

    in_maps = []
    for c in range(NCORES):
        in_maps.append({"xs": _pack_shard(x_bf, CW * c), "tmat": tmat})

    nc = _build_program(float(bias[0]))

    trace = bool(int(os.environ.get("CONV_KERNEL_TRACE", "0")))
    res = run_bass_kernel_spmd(nc, in_maps, core_ids=list(range(NCORES)),
                               trace=trace)
    if trace:
        kernel.last_exec_time_ns = res.exec_time_ns

    cols = []
    for c in range(NCORES):
        valid_out = min(CW, OW - CW * c)
        cols.append(_unpack_out(np.asarray(res.results[c]["y"]))[:, :valid_out])
    return np.concatenate(cols, axis=1).astype(np.float32)
